# revision 11
# baseline (speedup 1.0000x reference)
"""IoU loss kernel for Trainium2, data-parallel across 8 NeuronCores.

Math per box pair (reference semantics, see problem reference):
  u = x_o - x_t, v = y_o - y_t, s2 = s_o + s_t, sd = s_o - s_t
  sw = |s_o| + |s_t| = max(|s2|, |sd|)
  dw = ||s_o| - |s_t|| = min(|s2|, |sd|)
  |dx| = sw - max(dw, |u|)   (== middle-two sorted-x difference magnitude)
  |dy| = sw - max(dw, |v|)
  pr = |dx| * |dy|           (>= 0 whenever the disjoint flag is false)
  gate = (s2 >= max(|u|, |v|))   (NOT of the reference disjoint flag)
  ovg = pr * gate
  union = (2 s_o)^2 + (2 s_t)^2 - ovg
  q = ovg / (union + EPS)
  loss = sum(1 - q) = N - sum(q)

Host side: shard rows over 8 cores, de-interleave [R,3] -> channel-major
planes, cast to bf16, pack as [CHUNKS, 128, 6F] per core. Device computes
sum(q) partials via DVE/ACT elementwise pipeline + PE ones-matmul reduce.
"""

import os

import numpy as np

import concourse.bass as bass
import concourse.bacc as bacc
import concourse.mybir as mybir
from concourse.tile import TileContext
from concourse.bass_utils import run_bass_kernel_spmd

Alu = mybir.AluOpType
Act = mybir.ActivationFunctionType
BF16 = mybir.dt.bfloat16
F32 = mybir.dt.float32

N_CORES = 8
N_TOTAL = 4194304
R = N_TOTAL // N_CORES  # rows per core
P = 128
F = 1024  # rows per partition per chunk
CHUNKS = R // (P * F)
EPS = 1e-7

# Toggled by test.py for profiling runs.
PROFILE = False
LAST_EXEC_NS = None

# walrus CoreV3 codegen rejects AluOpType.abs_max; keep the ACT-abs fallback.
USE_ABSMAX = False
USE_ISGE = True

_BUILD_CACHE = {}


def _build_nc(chunks: int, f: int):
    """Build the per-core Bass program (SPMD: same program, per-core data)."""
    nc = bacc.Bacc("TRN2", target_bir_lowering=False, debug=False)
    comb = nc.dram_tensor("comb", [chunks, P, 6 * f], BF16, kind="ExternalInput")
    part = nc.dram_tensor("partial", [1, 512], F32, kind="ExternalOutput")

    n_mm = f // 512  # matmuls per chunk (PE moving free-dim cap is 512)

    with TileContext(nc) as tc:
        with (
            tc.tile_pool(name="io", bufs=3) as io_pool,
            tc.tile_pool(name="work", bufs=2) as wk,
            tc.tile_pool(name="const", bufs=1) as cpool,
            tc.tile_pool(name="psum", bufs=1, space="PSUM") as pp,
        ):
            ones = cpool.tile([P, 1], BF16, tag="ones")
            nc.vector.memset(ones, 1.0)
            psum = pp.tile([1, 512], F32, tag="acc")

            for k in range(chunks):
                t = io_pool.tile([P, 6 * f], BF16, tag="comb")
                nc.sync.dma_start(out=t, in_=comb.ap()[k])
                S, TS = t[:, 2 * f : 3 * f], t[:, 5 * f : 6 * f]

                # [u | v | sd] in one op, then s2.
                uvss = wk.tile([P, 4 * f], BF16, tag="uvss")
                nc.vector.tensor_sub(
                    out=uvss[:, 0 : 3 * f], in0=t[:, 0 : 3 * f], in1=t[:, 3 * f : 6 * f]
                )
                nc.vector.tensor_add(out=uvss[:, 3 * f : 4 * f], in0=S, in1=TS)
                u, v = uvss[:, 0:f], uvss[:, f : 2 * f]
                s2 = uvss[:, 3 * f : 4 * f]

                # |sd|, |s2| in one ACT pass (contiguous halves of uvss).
                aa = wk.tile([P, 2 * f], BF16, tag="aa")
                nc.scalar.activation(aa, uvss[:, 2 * f : 4 * f], Act.Abs)
                asd, as2 = aa[:, 0:f], aa[:, f : 2 * f]

                sw = wk.tile([P, f], BF16, tag="sw")
                nc.vector.tensor_tensor(out=sw, in0=as2, in1=asd, op=Alu.max)
                dw = wk.tile([P, f], BF16, tag="dw")
                nc.vector.tensor_tensor(out=dw, in0=as2, in1=asd, op=Alu.min)

                if USE_ABSMAX:
                    au, av = u, v
                    mxop = Alu.abs_max
                else:
                    auav = wk.tile([P, 2 * f], BF16, tag="auav")
                    nc.scalar.activation(auav, uvss[:, 0 : 2 * f], Act.Abs)
                    au, av = auav[:, 0:f], auav[:, f : 2 * f]
                    mxop = Alu.max
                mx = wk.tile([P, f], BF16, tag="mx")
                nc.vector.tensor_tensor(out=mx, in0=dw, in1=au, op=mxop)
                my = wk.tile([P, f], BF16, tag="my")
                nc.vector.tensor_tensor(out=my, in0=dw, in1=av, op=mxop)

                # dx = sw - mx, dy = sw - my  (fused as (mx * -1) + sw)
                dx = wk.tile([P, f], BF16, tag="dx")
                nc.vector.scalar_tensor_tensor(
                    out=dx, in0=mx, scalar=-1.0, in1=sw, op0=Alu.mult, op1=Alu.add
                )
                dy = wk.tile([P, f], BF16, tag="dy")
                nc.vector.scalar_tensor_tensor(
                    out=dy, in0=my, scalar=-1.0, in1=sw, op0=Alu.mult, op1=Alu.add
                )

                pr = wk.tile([P, f], BF16, tag="pr")
                nc.vector.tensor_mul(out=pr, in0=dx, in1=dy)

                mxy = wk.tile([P, f], BF16, tag="mxy")
                nc.vector.tensor_tensor(out=mxy, in0=au, in1=av, op=mxop)
                gate = wk.tile([P, f], BF16, tag="gate")
                if USE_ISGE:
                    nc.vector.tensor_tensor(out=gate, in0=s2, in1=mxy, op=Alu.is_ge)
                else:
                    # gate = relu(sign(s2 - mxy)); sign(0)=0 flips exact-tie
                    # elements (measure-zero w/ random f32, and q→0 there for
                    # proper boxes).
                    gd = wk.tile([P, f], BF16, tag="gd")
                    nc.vector.tensor_sub(out=gd, in0=s2, in1=mxy)
                    sg = wk.tile([P, f], BF16, tag="sg")
                    nc.scalar.activation(sg, gd, Act.Sign)
                    nc.scalar.activation(gate, sg, Act.Relu)
                ovg = wk.tile([P, f], BF16, tag="ovg")
                nc.vector.tensor_mul(out=ovg, in0=pr, in1=gate)

                # Areas (2 s)^2 via ACT Square with scale=2, reading raw tile.
                areas = wk.tile([P, 2 * f], BF16, tag="areas")
                nc.scalar.activation(areas[:, 0:f], S, Act.Square, scale=2.0)
                nc.scalar.activation(areas[:, f : 2 * f], TS, Act.Square, scale=2.0)
                sum2 = wk.tile([P, f], BF16, tag="sum2")
                nc.vector.tensor_add(out=sum2, in0=areas[:, 0:f], in1=areas[:, f : 2 * f])

                # un = (sum2 + EPS) - ovg, f32 (engine-internal fp32 keeps EPS).
                un = wk.tile([P, f], F32, tag="un")
                nc.vector.scalar_tensor_tensor(
                    out=un, in0=sum2, scalar=EPS, in1=ovg, op0=Alu.add, op1=Alu.subtract
                )
                rcp = wk.tile([P, f], F32, tag="rcp")
                nc.vector.reciprocal_approx_fast(rcp, un)

                q = wk.tile([P, f], BF16, tag="q")
                nc.vector.tensor_mul(out=q, in0=ovg, in1=rcp)

                # Partition-axis sum via PE: ones[128,1]^T @ q -> psum[1,512],
                # accumulating across subtiles and chunks.
                for m in range(n_mm):
                    nc.tensor.matmul(
                        psum,
                        ones,
                        q[:, 512 * m : 512 * (m + 1)],
                        start=(k == 0 and m == 0),
                        stop=(k == chunks - 1 and m == n_mm - 1),
                    )

            outt = cpool.tile([1, 512], F32, tag="outt")
            nc.vector.tensor_copy(out=outt, in_=psum)
            nc.sync.dma_start(out=part.ap(), in_=outt)
    nc.compile()
    return nc


def _get_nc(chunks: int, f: int):
    key = (chunks, f)
    if key not in _BUILD_CACHE:
        _BUILD_CACHE[key] = _build_nc(chunks, f)
    return _BUILD_CACHE[key]


def _pack_core(out_rows: np.ndarray, tar_rows: np.ndarray, chunks: int, f: int):
    """[R,3] f32 x2 -> [chunks, 128, 6F] bf16 channel-major pack."""
    o = out_rows.reshape(chunks, P, f, 3).transpose(0, 1, 3, 2)  # [c,P,3,F]
    t = tar_rows.reshape(chunks, P, f, 3).transpose(0, 1, 3, 2)
    packed = np.concatenate([o, t], axis=2)  # [c,P,6,F]
    import ml_dtypes

    return np.ascontiguousarray(packed.reshape(chunks, P, 6 * f)).astype(
        ml_dtypes.bfloat16
    )


def _ensure_ntff_hook():
    """Provide antenv.axon_hooks (missing in this image) so trace=True can
    capture NTFF profiles via the axon .so ctypes path from trn_boot."""
    import sys
    import types

    try:
        from antenv.axon_hooks import get_axon_ntff_profile_hook  # noqa: F401

        return
    except ImportError:
        pass
    try:
        boot_dir = "/root/.axon_site/trn_agent_boot"
        if boot_dir not in sys.path:
            sys.path.insert(0, boot_dir)
        import trn_boot  # type: ignore

        hook = trn_boot._ntff_profile_via_ctypes("/opt/axon/libaxon_pjrt.so")
        mod = types.ModuleType("antenv.axon_hooks")
        mod.get_axon_ntff_profile_hook = lambda: hook
        mod.set_axon_ntff_profile_hook = lambda h: None
        sys.modules["antenv.axon_hooks"] = mod
    except Exception:
        pass


def kernel(outputs: np.ndarray, targets: np.ndarray) -> np.ndarray:
    global LAST_EXEC_NS
    if PROFILE:
        _ensure_ntff_hook()
    outputs = np.asarray(outputs, dtype=np.float32)
    targets = np.asarray(targets, dtype=np.float32)
    n = outputs.shape[0]
    assert n == N_TOTAL, f"kernel hardcoded for N={N_TOTAL}, got {n}"

    nc = _get_nc(CHUNKS, F)
    in_maps = []
    for c in range(N_CORES):
        sl = slice(c * R, (c + 1) * R)
        in_maps.append({"comb": _pack_core(outputs[sl], targets[sl], CHUNKS, F)})

    res = run_bass_kernel_spmd(
        nc,
        in_maps,
        core_ids=list(range(N_CORES)),
        trace=PROFILE,
    )
    LAST_EXEC_NS = res.exec_time_ns

    total_q = np.sum(
        np.stack([r["partial"].astype(np.float64) for r in res.results])
    )
    loss = np.float32(np.float64(n) - total_q)
    return np.asarray(loss, dtype=np.float32)
